# revision 13
# baseline (speedup 1.0000x reference)
"""DeltaHebbianBlock Trainium2 kernel.

Sharding: 8 cores = (B=2) x (H=4) head-parallel. Each core computes its
head's delta-rule chunked scan and the partial output projection
partial_bh = (alpha_h * o_bh) @ Wr_h^T  (8192 x 1024).
Host gathers: out[b] = x[b] + sum_h partial[b,h].

Per-core pipeline (T=8192, d=256, C=64, 128 chunks, 8 quarter-passes):
  P1: DMA-transpose x -> xT (bf16), v = x @ WwT (bf16 mm, f32 psum),
      rk = normalize(x_h), rkT via PE transpose, wk = shift(rk) via SBUF DMA.
  P2: per chunk-pair (block-diag 128x128): grams W = wk wk^T, intraT;
      A^T = (I+C0)(I+C1)(I+C2) truncated nilpotent chain (exact to A0^7);
      v_corr = A v, wk_corrT = (A wk)^T; rkgT, wkgN scalings.
  P3: sequential scan: v_new = v_corr - wk_corr S; o = rkg S + intra v_new;
      S = gC S + wkgN^T v_new.
  P4: oT via PE transpose; partial = oT^T @ (alpha WrT) (bf16 mm).
"""
import os
import numpy as np
import ml_dtypes
from contextlib import ExitStack

import concourse.bass as bass
import concourse.mybir as mybir
import concourse.tile as tile
from concourse import bacc, bass_utils

B, T, D = 2, 8192, 1024
H, d, C = 4, 256, 64
NCH = T // C          # 128 chunks
NQ = 8                # quarter passes
QT = T // NQ          # 1024 tokens per pass
QTT = QT // 128       # 8 p-tiles per pass
QCH = QT // C         # 16 chunks per pass
QPR = QCH // 2        # 8 pairs per pass

F32 = mybir.dt.float32
BF16 = mybir.dt.bfloat16


def _build():
    nc = bacc.Bacc("TRN2", target_bir_lowering=False, debug=False, num_devices=int(os.environ.get("K_NCORES", "8")))
    xbf = nc.dram_tensor("xbf", (T, D), BF16, kind="ExternalInput")
    wwt = nc.dram_tensor("wwt", (D, d), BF16, kind="ExternalInput")
    wrt = nc.dram_tensor("wrt", (d, D), BF16, kind="ExternalInput")
    mb_d = nc.dram_tensor("mb", (128, 128), F32, kind="ExternalInput")
    mc_d = nc.dram_tensor("mc", (128, 128), F32, kind="ExternalInput")
    mit_d = nc.dram_tensor("mit", (128, 128), F32, kind="ExternalInput")
    id_d = nc.dram_tensor("ident", (128, 128), BF16, kind="ExternalInput")
    gpb_d = nc.dram_tensor("gpbf", (128, QT), BF16, kind="ExternalInput")
    gpt_d = nc.dram_tensor("gpt", (128, 1), F32, kind="ExternalInput")
    gcv_d = nc.dram_tensor("gcv", (128, 1), F32, kind="ExternalInput")
    part_d = nc.dram_tensor("partial", (T, D), F32, kind="ExternalOutput")

    with ExitStack() as ctx:
        tc = ctx.enter_context(tile.TileContext(nc))
        consts = ctx.enter_context(tc.tile_pool(name="consts", bufs=1))
        big = ctx.enter_context(tc.tile_pool(name="big", bufs=1))
        qbuf = ctx.enter_context(tc.tile_pool(name="qbuf", bufs=1))
        chain = ctx.enter_context(tc.tile_pool(name="chain", bufs=2))
        vnewp = ctx.enter_context(tc.tile_pool(name="vnewp", bufs=3))
        stage = ctx.enter_context(tc.tile_pool(name="stage", bufs=3))
        scr = ctx.enter_context(tc.tile_pool(name="scr", bufs=2))
        ps_g = ctx.enter_context(tc.tile_pool(name="ps_g", bufs=2, space="PSUM"))
        ps_a = ctx.enter_context(tc.tile_pool(name="ps_a", bufs=2, space="PSUM"))
        ps_s = ctx.enter_context(tc.tile_pool(name="ps_s", bufs=2, space="PSUM"))
        ps_p = ctx.enter_context(tc.tile_pool(name="ps_p", bufs=2, space="PSUM"))

        # ---- constants / weights in SBUF ----
        wwt_s = consts.tile([128, 8, d], BF16)
        nc.sync.dma_start(wwt_s[:], wwt.ap().rearrange("(kb p) j -> p kb j", p=128))
        wrt_s = consts.tile([128, 2, D], BF16)
        nc.sync.dma_start(wrt_s[:], wrt.ap().rearrange("(kt p) n -> p kt n", p=128))
        mb_s = consts.tile([128, 128], F32)
        nc.sync.dma_start(mb_s[:], mb_d.ap())
        mc_s = consts.tile([128, 128], F32)
        nc.sync.dma_start(mc_s[:], mc_d.ap())
        mit_s = consts.tile([128, 128], F32)
        nc.sync.dma_start(mit_s[:], mit_d.ap())
        id_s = consts.tile([128, 128], BF16)
        nc.sync.dma_start(id_s[:], id_d.ap())
        gpb_s = consts.tile([128, QT], BF16)
        nc.sync.dma_start(gpb_s[:], gpb_d.ap())
        gpt_s = consts.tile([128, 1], F32)
        nc.sync.dma_start(gpt_s[:], gpt_d.ap())
        gcv_s = consts.tile([128, 1], F32)
        nc.sync.dma_start(gcv_s[:], gcv_d.ap())

        # ---- full-T persistent (bf16) ----
        rk = big.tile([128, T // 128, d], BF16)       # 4MB
        wk = big.tile([128, T // 128, d], BF16)       # 4MB
        rkT = big.tile([128, 2, T + 1], BF16)         # 4MB (col 0 = zero pad)
        S_bf = big.tile([128, 2, d], BF16)
        nc.gpsimd.memset(S_bf[:], 0.0)
        nc.gpsimd.memset(rkT[:, :, 0:1], 0.0)
        nc.gpsimd.memset(wk[0:1, 0:1, :], 0.0)

        for q in range(NQ):
            if os.environ.get("K_STOP") == "consts":
                break
            qt0 = q * QT          # token offset
            tt0 = q * QTT         # p-tile offset
            # ---------------- P1 ----------------
            xT = qbuf.tile([128, 8, QT], BF16, tag="xT")
            for kb in range(8):
                nc.sync.dma_start(
                    xT[:, kb, :],
                    xbf.ap()[qt0:qt0 + QT, kb * 128:(kb + 1) * 128],
                    transpose=True)
            if os.environ.get("K_STOP") == "xt":
                continue
            xh = qbuf.tile([128, QTT, d], BF16, tag="xh")
            h_ap = xbf.ap()[qt0:qt0 + QT, :]  # head slice set on host via col offset 0
            nc.sync.dma_start(
                xh[:], h_ap[:, 0:d].rearrange("(tt p) j -> p tt j", p=128))
            if os.environ.get("K_STOP") == "xh":
                continue
            v_nat = qbuf.tile([128, QTT, d], BF16, tag="v_nat")
            for tt in range(QTT):
                vps = ps_p.tile([128, d], F32, tag="p")
                nkb = int(os.environ.get("K_KB", "8"))
                for kb in range(nkb):
                    nc.tensor.matmul(vps[:], xT[:, kb, tt * 128:(tt + 1) * 128],
                                     wwt_s[:, kb, :], start=(kb == 0), stop=(kb == nkb - 1))
                nc.vector.tensor_copy(v_nat[:, tt, :], vps[:])
            if os.environ.get("K_STOP") == "v":
                continue
            # rk = normalize(xh)
            rklvl = os.environ.get("K_RK", "all")
            for tt in range(QTT):
                sq = scr.tile([128, d], F32, tag="sq")
                ss = scr.tile([128, 1], F32, tag="ss")
                nc.scalar.activation(sq[:], xh[:, tt, :],
                                     mybir.ActivationFunctionType.Square,
                                     accum_out=ss[:])
                if rklvl == "red":
                    continue
                nrm = scr.tile([128, 1], F32, tag="nrm")
                nc.scalar.activation(nrm[:], ss[:], mybir.ActivationFunctionType.Sqrt)
                inv = scr.tile([128, 1], F32, tag="inv")
                nc.vector.reciprocal(inv[:], nrm[:])
                if rklvl == "sqrt":
                    continue
                nc.scalar.activation(rk[:, tt0 + tt, :], xh[:, tt, :],
                                     mybir.ActivationFunctionType.Copy, scale=inv[:])
                if rklvl == "scale":
                    continue
                for kt in range(2):
                    tps = ps_g.tile([128, 128], BF16, tag="g")
                    nc.tensor.transpose(tps[:], rk[:, tt0 + tt, kt * 128:(kt + 1) * 128],
                                        id_s[:])
                    nc.vector.tensor_copy(
                        rkT[:, kt, 1 + qt0 + tt * 128: 1 + qt0 + (tt + 1) * 128], tps[:])
            if os.environ.get("K_STOP") == "rk":
                continue
            # wk = shift(rk) by one row
            nc.sync.dma_start(wk[1:128, tt0:tt0 + QTT, :], rk[0:127, tt0:tt0 + QTT, :])
            lo = max(tt0, 1)
            nc.sync.dma_start(wk[0:1, lo:tt0 + QTT, :], rk[127:128, lo - 1:tt0 + QTT - 1, :])
            # wkgN = wk * gp_tail (per-partition), rkgT = rkT * gp (per-col)
            wkgN = qbuf.tile([128, QTT, d], BF16, tag="wkgN")
            nc.scalar.activation(wkgN[:], wk[:, tt0:tt0 + QTT, :],
                                 mybir.ActivationFunctionType.Copy, scale=gpt_s[:])
            rkgT = qbuf.tile([128, 2, QT], BF16, tag="rkgT")
            for kt in range(2):
                nc.vector.tensor_mul(rkgT[:, kt, :], rkT[:, kt, 1 + qt0:1 + qt0 + QT],
                                     gpb_s[:])
            if os.environ.get("K_STOP") == "p1":
                continue
            # ---------------- P2 ----------------
            AT = qbuf.tile([128, QPR * 128], BF16, tag="AT")
            inT = qbuf.tile([128, QPR * 128], BF16, tag="inT")
            v_corr = qbuf.tile([128, QTT, d], BF16, tag="v_corr")
            wkcT = qbuf.tile([128, 2, QT], BF16, tag="wkcT")
            for p in range(QPR):
                w0 = qt0 + p * 128  # global token col of pair window
                gps = ps_g.tile([128, 128], F32, tag="g")
                for kt in range(2):
                    nc.tensor.matmul(gps[:], rkT[:, kt, w0:w0 + 128],
                                     rkT[:, kt, w0:w0 + 128],
                                     start=(kt == 0), stop=(kt == 1))
                B0 = chain.tile([128, 128], BF16, tag="B0")
                nc.vector.tensor_mul(B0[:], gps[:], mb_s[:])
                C0 = chain.tile([128, 128], BF16, tag="C0")
                nc.vector.tensor_mul(C0[:], gps[:], mc_s[:])
                ips = ps_g.tile([128, 128], F32, tag="g")
                for kt in range(2):
                    nc.tensor.matmul(ips[:], rkT[:, kt, w0:w0 + 128],
                                     rkT[:, kt, w0 + 1:w0 + 129],
                                     start=(kt == 0), stop=(kt == 1))
                nc.vector.tensor_mul(inT[:, p * 128:(p + 1) * 128], ips[:], mit_s[:])
                # chain: C1 = C0^2, B1 = C1^T-path, C2 = C1^2
                c1p = ps_g.tile([128, 128], F32, tag="g")
                nc.tensor.matmul(c1p[:], B0[:], C0[:])
                C1 = chain.tile([128, 128], BF16, tag="C1")
                nc.vector.tensor_copy(C1[:], c1p[:])
                b1p = ps_g.tile([128, 128], F32, tag="g")
                nc.tensor.matmul(b1p[:], C0[:], B0[:])
                B1 = chain.tile([128, 128], BF16, tag="B1")
                nc.vector.tensor_copy(B1[:], b1p[:])
                c2p = ps_g.tile([128, 128], F32, tag="g")
                nc.tensor.matmul(c2p[:], B1[:], C1[:])
                C2 = chain.tile([128, 128], BF16, tag="C2")
                nc.vector.tensor_copy(C2[:], c2p[:])
                G0 = chain.tile([128, 128], BF16, tag="G0")
                nc.vector.tensor_add(G0[:], B0[:], id_s[:])
                g1p = ps_g.tile([128, 128], F32, tag="g")
                nc.tensor.matmul(g1p[:], G0[:], C1[:], start=True, stop=False)
                nc.tensor.matmul(g1p[:], G0[:], id_s[:], start=False, stop=True)
                G1h = chain.tile([128, 128], BF16, tag="G1h")
                nc.vector.tensor_copy(G1h[:], g1p[:])
                g1tp = ps_g.tile([128, 128], BF16, tag="g")
                nc.tensor.transpose(g1tp[:], G1h[:], id_s[:])
                G1 = chain.tile([128, 128], BF16, tag="G1")
                nc.vector.tensor_copy(G1[:], g1tp[:])
                g2p = ps_g.tile([128, 128], F32, tag="g")
                nc.tensor.matmul(g2p[:], G1[:], C2[:], start=True, stop=False)
                nc.tensor.matmul(g2p[:], G1[:], id_s[:], start=False, stop=True)
                nc.vector.tensor_copy(AT[:, p * 128:(p + 1) * 128], g2p[:])
                # applications
                vcp = ps_a.tile([128, d], F32, tag="a")
                nc.tensor.matmul(vcp[:], AT[:, p * 128:(p + 1) * 128], v_nat[:, p, :])
                nc.vector.tensor_copy(v_corr[:, p, :], vcp[:])
                for jb in range(2):
                    wcp = ps_a.tile([128, 128], F32, tag="a")
                    nc.tensor.matmul(wcp[:], wk[:, tt0 + p, jb * 128:(jb + 1) * 128],
                                     AT[:, p * 128:(p + 1) * 128])
                    nc.vector.tensor_copy(wkcT[:, jb, p * 128:(p + 1) * 128], wcp[:])
            if os.environ.get("K_STOP") == "p2":
                continue
            # ---------------- P3: scan ----------------
            o_nat = qbuf.tile([128, QTT, d], BF16, tag="o_nat")
            for cq in range(QCH):
                tt = cq // 2
                poff = (cq % 2) * 64
                gcol = cq * 64
                p = cq // 2
                sl = slice(poff, poff + 64)
                vnp = ps_a.tile([128, d], F32, tag="a")
                for jb in range(2):
                    nc.tensor.matmul(vnp[sl, :], wkcT[:, jb, gcol:gcol + 64], S_bf[:, jb, :],
                                     start=(jb == 0), stop=(jb == 1))
                vnew = vnewp.tile([128, d], BF16, tag="vnew")
                nc.vector.scalar_tensor_tensor(
                    vnew[sl, :], vnp[sl, :], -1.0, v_corr[sl, tt, :],
                    mybir.AluOpType.mult, mybir.AluOpType.add)
                ops = ps_a.tile([128, d], F32, tag="a")
                for jb in range(2):
                    nc.tensor.matmul(ops[sl, :], rkgT[:, jb, gcol:gcol + 64], S_bf[:, jb, :],
                                     start=(jb == 0), stop=False)
                nc.tensor.matmul(ops[sl, :], inT[sl, p * 128 + poff:p * 128 + poff + 64],
                                 vnew[sl, :], start=False, stop=True)
                nc.vector.tensor_copy(o_nat[sl, tt, :], ops[sl, :])
                for jb in range(2):
                    sup = ps_s.tile([128, d], F32, tag="s")
                    nc.tensor.matmul(sup[:], wkgN[sl, tt, jb * 128:(jb + 1) * 128],
                                     vnew[sl, :])
                    nc.vector.scalar_tensor_tensor(
                        S_bf[:, jb, :], S_bf[:, jb, :], gcv_s[:], sup[:],
                        mybir.AluOpType.mult, mybir.AluOpType.add)
            if os.environ.get("K_STOP") == "p3":
                continue
            # ---------------- P4 ----------------
            oT = qbuf.tile([128, 2, QT], BF16, tag="oT")
            for p in range(QPR):
                for kt in range(2):
                    otp = ps_g.tile([128, 128], BF16, tag="g")
                    nc.tensor.transpose(otp[:], o_nat[:, p, kt * 128:(kt + 1) * 128], id_s[:])
                    nc.vector.tensor_copy(oT[:, kt, p * 128:(p + 1) * 128], otp[:])
                st = stage.tile([128, D], F32, tag="st")
                for nh in range(2):
                    pps = ps_p.tile([128, 512], F32, tag="p")
                    for kt in range(2):
                        nc.tensor.matmul(pps[:], oT[:, kt, p * 128:(p + 1) * 128],
                                         wrt_s[:, kt, nh * 512:(nh + 1) * 512],
                                         start=(kt == 0), stop=(kt == 1))
                    nc.vector.tensor_copy(st[:, nh * 512:(nh + 1) * 512], pps[:])
                nc.sync.dma_start(
                    part_d.ap()[qt0 + p * 128: qt0 + (p + 1) * 128, :], st[:])
    nc.compile()
    return nc


_NC = None
LAST_EXEC_NS = None
LAST_TRACE = None


def _bf16(a):
    return np.ascontiguousarray(a.astype(ml_dtypes.bfloat16))


def kernel(out, Ww, Wr, decay, log_alpha):
    global _NC
    out = np.asarray(out, dtype=np.float32)
    Ww = np.asarray(Ww, dtype=np.float32)
    Wr = np.asarray(Wr, dtype=np.float32)
    decay = np.asarray(decay, dtype=np.float32)
    log_alpha = np.asarray(log_alpha, dtype=np.float32)
    gamma = 1.0 / (1.0 + np.exp(-decay.astype(np.float64)))
    alpha = np.exp(log_alpha.astype(np.float64))

    if _NC is None:
        _NC = _build()
    nc = _NC

    p64 = np.arange(64)
    in_maps = []
    for c in range(8):
        b, h = c // 4, c % 4
        g = gamma[h]
        # x with head-h channels rotated to the front so the kernel's
        # xh slice [:, 0:d] is the head slice (v-proj uses matching
        # rotated WwT so the product is unchanged).
        xr = np.roll(out[b], -h * d, axis=1)
        wwr = np.roll(Ww[h * d:(h + 1) * d, :], -h * d, axis=1).T  # (D, d)
        wrs = (alpha[h] * Wr[:, h * d:(h + 1) * d]).T              # (d, D)
        Ls = np.tril(g ** np.maximum(p64[:, None] - p64[None, :], 0), -1)
        mbB = (-Ls).astype(np.float32)
        mitB = np.triu(g ** np.maximum(p64[None, :] - p64[:, None], 0), 1).astype(np.float32)
        z = np.zeros((64, 64), np.float32)
        mb = np.block([[mbB, z], [z, mbB]])
        mit = np.block([[mitB, z], [z, mitB]])
        gp = (g ** p64).astype(np.float32)
        gpb = np.tile(gp, QT // 64)[None, :].repeat(128, 0)
        gpt = (g ** (63 - (np.arange(128) % 64)))[:, None].astype(np.float32)
        gcv = np.full((128, 1), g ** 64, np.float32)
        in_maps.append({
            "xbf": _bf16(xr),
            "wwt": _bf16(wwr),
            "wrt": _bf16(wrs),
            "mb": mb, "mc": np.ascontiguousarray(mb.T),
            "mit": mit,
            "ident": _bf16(np.eye(128, dtype=np.float32)),
            "gpbf": _bf16(gpb),
            "gpt": gpt, "gcv": gcv,
        })

    ncore = int(os.environ.get("K_NCORES", "8"))
    res = bass_utils.run_bass_kernel_spmd(
        nc, in_maps[:ncore], core_ids=list(range(ncore)),
        trace=bool(os.environ.get("K_TRACE")))
    global LAST_EXEC_NS, LAST_TRACE
    LAST_EXEC_NS = res.exec_time_ns
    LAST_TRACE = res.instructions_and_trace
    final = out.copy()
    for c in range(len(res.results)):
        b = c // 4
        final[b] += res.results[c]["partial"]
    return final


# revision 14
# speedup vs baseline: 1.0035x; 1.0035x over previous
"""DeltaHebbianBlock Trainium2 kernel.

Sharding: 8 cores = (B=2) x (H=4) head-parallel. Each core computes its
head's delta-rule chunked scan and the partial output projection
partial_bh = (alpha_h * o_bh) @ Wr_h^T  (8192 x 1024).
Host gathers: out[b] = x[b] + sum_h partial[b,h].

Per-core pipeline (T=8192, d=256, C=64, 128 chunks, 8 quarter-passes):
  P1: DMA-transpose x -> xT (bf16), v = x @ WwT (bf16 mm, f32 psum),
      rk = normalize(x_h), rkT via PE transpose, wk = shift(rk) via SBUF DMA.
  P2: per chunk-pair (block-diag 128x128): grams W = wk wk^T, intraT;
      A^T = (I+C0)(I+C1)(I+C2) truncated nilpotent chain (exact to A0^7);
      v_corr = A v, wk_corrT = (A wk)^T; rkgT, wkgN scalings.
  P3: sequential scan: v_new = v_corr - wk_corr S; o = rkg S + intra v_new;
      S = gC S + wkgN^T v_new.
  P4: oT via PE transpose; partial = oT^T @ (alpha WrT) (bf16 mm).
"""
import os
import numpy as np
import ml_dtypes
from contextlib import ExitStack

import concourse.bass as bass
import concourse.mybir as mybir
import concourse.tile as tile
from concourse import bacc, bass_utils

B, T, D = 2, 8192, 1024
H, d, C = 4, 256, 64
NCH = T // C          # 128 chunks
NQ = 8                # quarter passes
QT = T // NQ          # 1024 tokens per pass
QTT = QT // 128       # 8 p-tiles per pass
QCH = QT // C         # 16 chunks per pass
QPR = QCH // 2        # 8 pairs per pass

F32 = mybir.dt.float32
BF16 = mybir.dt.bfloat16


def _build():
    nc = bacc.Bacc("TRN2", target_bir_lowering=False, debug=False, num_devices=int(os.environ.get("K_NCORES", "8")))
    xbf = nc.dram_tensor("xbf", (T, D), BF16, kind="ExternalInput")
    wwt = nc.dram_tensor("wwt", (D, d), BF16, kind="ExternalInput")
    wrt = nc.dram_tensor("wrt", (d, D), BF16, kind="ExternalInput")
    mb_d = nc.dram_tensor("mb", (128, 128), F32, kind="ExternalInput")
    mc_d = nc.dram_tensor("mc", (128, 128), F32, kind="ExternalInput")
    mit_d = nc.dram_tensor("mit", (128, 128), F32, kind="ExternalInput")
    id_d = nc.dram_tensor("ident", (128, 128), BF16, kind="ExternalInput")
    gpb_d = nc.dram_tensor("gpbf", (128, QT), BF16, kind="ExternalInput")
    gpt_d = nc.dram_tensor("gpt", (128, 1), F32, kind="ExternalInput")
    gcv_d = nc.dram_tensor("gcv", (128, 1), F32, kind="ExternalInput")
    part_d = nc.dram_tensor("partial", (T, D), F32, kind="ExternalOutput")

    with ExitStack() as ctx:
        tc = ctx.enter_context(tile.TileContext(nc))
        consts = ctx.enter_context(tc.tile_pool(name="consts", bufs=1))
        big = ctx.enter_context(tc.tile_pool(name="big", bufs=1))
        qbuf = ctx.enter_context(tc.tile_pool(name="qbuf", bufs=1))
        qbuf2 = ctx.enter_context(tc.tile_pool(name="qbuf2", bufs=2))
        chain = ctx.enter_context(tc.tile_pool(name="chain", bufs=2))
        vnewp = ctx.enter_context(tc.tile_pool(name="vnewp", bufs=3))
        stage = ctx.enter_context(tc.tile_pool(name="stage", bufs=3))
        scr = ctx.enter_context(tc.tile_pool(name="scr", bufs=2))
        ps_g = ctx.enter_context(tc.tile_pool(name="ps_g", bufs=2, space="PSUM"))
        ps_a = ctx.enter_context(tc.tile_pool(name="ps_a", bufs=2, space="PSUM"))
        ps_s = ctx.enter_context(tc.tile_pool(name="ps_s", bufs=2, space="PSUM"))
        ps_p = ctx.enter_context(tc.tile_pool(name="ps_p", bufs=2, space="PSUM"))

        # ---- constants / weights in SBUF ----
        wwt_s = consts.tile([128, 8, d], BF16)
        nc.sync.dma_start(wwt_s[:], wwt.ap().rearrange("(kb p) j -> p kb j", p=128))
        wrt_s = consts.tile([128, 2, D], BF16)
        nc.sync.dma_start(wrt_s[:], wrt.ap().rearrange("(kt p) n -> p kt n", p=128))
        mb_s = consts.tile([128, 128], F32)
        nc.sync.dma_start(mb_s[:], mb_d.ap())
        mc_s = consts.tile([128, 128], F32)
        nc.sync.dma_start(mc_s[:], mc_d.ap())
        mit_s = consts.tile([128, 128], F32)
        nc.sync.dma_start(mit_s[:], mit_d.ap())
        id_s = consts.tile([128, 128], BF16)
        nc.sync.dma_start(id_s[:], id_d.ap())
        gpb_s = consts.tile([128, QT], BF16)
        nc.sync.dma_start(gpb_s[:], gpb_d.ap())
        gpt_s = consts.tile([128, 1], F32)
        nc.sync.dma_start(gpt_s[:], gpt_d.ap())
        gcv_s = consts.tile([128, 1], F32)
        nc.sync.dma_start(gcv_s[:], gcv_d.ap())

        # ---- full-T persistent (bf16) ----
        rk = big.tile([128, T // 128, d], BF16)       # 4MB
        wk = big.tile([128, T // 128, d], BF16)       # 4MB
        rkT = big.tile([128, 2, T + 1], BF16)         # 4MB (col 0 = zero pad)
        S_bf = big.tile([128, 2, d], BF16)
        nc.gpsimd.memset(S_bf[:], 0.0)
        nc.gpsimd.memset(rkT[:, :, 0:1], 0.0)
        nc.gpsimd.memset(wk[0:1, 0:1, :], 0.0)

        for q in range(NQ):
            if os.environ.get("K_STOP") == "consts":
                break
            qt0 = q * QT          # token offset
            tt0 = q * QTT         # p-tile offset
            # ---------------- P1 ----------------
            xT = qbuf2.tile([128, 8, QT], BF16, tag="xT")
            for kb in range(8):
                nc.sync.dma_start(
                    xT[:, kb, :],
                    xbf.ap()[qt0:qt0 + QT, kb * 128:(kb + 1) * 128],
                    transpose=True)
            if os.environ.get("K_STOP") == "xt":
                continue
            xh = qbuf.tile([128, QTT, d], BF16, tag="xh")
            h_ap = xbf.ap()[qt0:qt0 + QT, :]  # head slice set on host via col offset 0
            nc.sync.dma_start(
                xh[:], h_ap[:, 0:d].rearrange("(tt p) j -> p tt j", p=128))
            if os.environ.get("K_STOP") == "xh":
                continue
            v_nat = qbuf2.tile([128, QTT, d], BF16, tag="v_nat")
            for tt in range(QTT):
                vps = ps_p.tile([128, d], F32, tag="p")
                nkb = int(os.environ.get("K_KB", "8"))
                for kb in range(nkb):
                    nc.tensor.matmul(vps[:], xT[:, kb, tt * 128:(tt + 1) * 128],
                                     wwt_s[:, kb, :], start=(kb == 0), stop=(kb == nkb - 1))
                nc.vector.tensor_copy(v_nat[:, tt, :], vps[:])
            if os.environ.get("K_STOP") == "v":
                continue
            # rk = normalize(xh)
            rklvl = os.environ.get("K_RK", "all")
            for tt in range(QTT):
                sq = scr.tile([128, d], F32, tag="sq")
                ss = scr.tile([128, 1], F32, tag="ss")
                nc.scalar.activation(sq[:], xh[:, tt, :],
                                     mybir.ActivationFunctionType.Square,
                                     accum_out=ss[:])
                if rklvl == "red":
                    continue
                nrm = scr.tile([128, 1], F32, tag="nrm")
                nc.scalar.activation(nrm[:], ss[:], mybir.ActivationFunctionType.Sqrt)
                inv = scr.tile([128, 1], F32, tag="inv")
                nc.vector.reciprocal(inv[:], nrm[:])
                if rklvl == "sqrt":
                    continue
                nc.scalar.activation(rk[:, tt0 + tt, :], xh[:, tt, :],
                                     mybir.ActivationFunctionType.Copy, scale=inv[:])
                if rklvl == "scale":
                    continue
                for kt in range(2):
                    tps = ps_g.tile([128, 128], BF16, tag="g")
                    nc.tensor.transpose(tps[:], rk[:, tt0 + tt, kt * 128:(kt + 1) * 128],
                                        id_s[:])
                    nc.vector.tensor_copy(
                        rkT[:, kt, 1 + qt0 + tt * 128: 1 + qt0 + (tt + 1) * 128], tps[:])
            if os.environ.get("K_STOP") == "rk":
                continue
            # wk = shift(rk) by one row
            nc.sync.dma_start(wk[1:128, tt0:tt0 + QTT, :], rk[0:127, tt0:tt0 + QTT, :])
            lo = max(tt0, 1)
            nc.sync.dma_start(wk[0:1, lo:tt0 + QTT, :], rk[127:128, lo - 1:tt0 + QTT - 1, :])
            # wkgN = wk * gp_tail (per-partition), rkgT = rkT * gp (per-col)
            wkgN = qbuf.tile([128, QTT, d], BF16, tag="wkgN")
            nc.scalar.activation(wkgN[:], wk[:, tt0:tt0 + QTT, :],
                                 mybir.ActivationFunctionType.Copy, scale=gpt_s[:])
            rkgT = qbuf.tile([128, 2, QT], BF16, tag="rkgT")
            for kt in range(2):
                nc.vector.tensor_mul(rkgT[:, kt, :], rkT[:, kt, 1 + qt0:1 + qt0 + QT],
                                     gpb_s[:])
            if os.environ.get("K_STOP") == "p1":
                continue
            # ---------------- P2 ----------------
            AT = qbuf.tile([128, QPR * 128], BF16, tag="AT")
            inT = qbuf.tile([128, QPR * 128], BF16, tag="inT")
            v_corr = qbuf.tile([128, QTT, d], BF16, tag="v_corr")
            wkcT = qbuf.tile([128, 2, QT], BF16, tag="wkcT")
            for p in range(QPR):
                w0 = qt0 + p * 128  # global token col of pair window
                gps = ps_g.tile([128, 128], F32, tag="g")
                for kt in range(2):
                    nc.tensor.matmul(gps[:], rkT[:, kt, w0:w0 + 128],
                                     rkT[:, kt, w0:w0 + 128],
                                     start=(kt == 0), stop=(kt == 1))
                B0 = chain.tile([128, 128], BF16, tag="B0")
                nc.vector.tensor_mul(B0[:], gps[:], mb_s[:])
                C0 = chain.tile([128, 128], BF16, tag="C0")
                nc.vector.tensor_mul(C0[:], gps[:], mc_s[:])
                ips = ps_g.tile([128, 128], F32, tag="g")
                for kt in range(2):
                    nc.tensor.matmul(ips[:], rkT[:, kt, w0:w0 + 128],
                                     rkT[:, kt, w0 + 1:w0 + 129],
                                     start=(kt == 0), stop=(kt == 1))
                nc.vector.tensor_mul(inT[:, p * 128:(p + 1) * 128], ips[:], mit_s[:])
                # chain: C1 = C0^2, B1 = C1^T-path, C2 = C1^2
                c1p = ps_g.tile([128, 128], F32, tag="g")
                nc.tensor.matmul(c1p[:], B0[:], C0[:])
                C1 = chain.tile([128, 128], BF16, tag="C1")
                nc.vector.tensor_copy(C1[:], c1p[:])
                b1p = ps_g.tile([128, 128], F32, tag="g")
                nc.tensor.matmul(b1p[:], C0[:], B0[:])
                B1 = chain.tile([128, 128], BF16, tag="B1")
                nc.vector.tensor_copy(B1[:], b1p[:])
                c2p = ps_g.tile([128, 128], F32, tag="g")
                nc.tensor.matmul(c2p[:], B1[:], C1[:])
                C2 = chain.tile([128, 128], BF16, tag="C2")
                nc.vector.tensor_copy(C2[:], c2p[:])
                G0 = chain.tile([128, 128], BF16, tag="G0")
                nc.vector.tensor_add(G0[:], B0[:], id_s[:])
                Gh0 = chain.tile([128, 128], BF16, tag="Gh0")
                nc.vector.tensor_add(Gh0[:], C0[:], id_s[:])
                g1p = ps_g.tile([128, 128], F32, tag="g")
                nc.tensor.matmul(g1p[:], G0[:], C1[:])
                G1h = chain.tile([128, 128], BF16, tag="G1h")
                nc.vector.tensor_add(G1h[:], g1p[:], Gh0[:])
                g1tp = ps_g.tile([128, 128], BF16, tag="g")
                nc.tensor.transpose(g1tp[:], G1h[:], id_s[:])
                G1 = chain.tile([128, 128], BF16, tag="G1")
                nc.vector.tensor_copy(G1[:], g1tp[:])
                g2p = ps_g.tile([128, 128], F32, tag="g")
                nc.tensor.matmul(g2p[:], G1[:], C2[:])
                nc.vector.tensor_add(AT[:, p * 128:(p + 1) * 128], g2p[:], G1h[:])
                # applications
                vcp = ps_a.tile([128, d], F32, tag="a")
                nc.tensor.matmul(vcp[:], AT[:, p * 128:(p + 1) * 128], v_nat[:, p, :])
                nc.vector.tensor_copy(v_corr[:, p, :], vcp[:])
                for jb in range(2):
                    wcp = ps_a.tile([128, 128], F32, tag="a")
                    nc.tensor.matmul(wcp[:], wk[:, tt0 + p, jb * 128:(jb + 1) * 128],
                                     AT[:, p * 128:(p + 1) * 128])
                    nc.vector.tensor_copy(wkcT[:, jb, p * 128:(p + 1) * 128], wcp[:])
            if os.environ.get("K_STOP") == "p2":
                continue
            # ---------------- P3: scan ----------------
            o_nat = qbuf2.tile([128, QTT, d], BF16, tag="o_nat")
            for cq in range(QCH):
                tt = cq // 2
                poff = (cq % 2) * 64
                gcol = cq * 64
                p = cq // 2
                sl = slice(poff, poff + 64)
                vnp = ps_a.tile([128, d], F32, tag="a")
                for jb in range(2):
                    nc.tensor.matmul(vnp[sl, :], wkcT[:, jb, gcol:gcol + 64], S_bf[:, jb, :],
                                     start=(jb == 0), stop=(jb == 1))
                vnew = vnewp.tile([128, d], BF16, tag="vnew")
                nc.vector.scalar_tensor_tensor(
                    vnew[sl, :], vnp[sl, :], -1.0, v_corr[sl, tt, :],
                    mybir.AluOpType.mult, mybir.AluOpType.add)
                ops = ps_a.tile([128, d], F32, tag="a")
                for jb in range(2):
                    nc.tensor.matmul(ops[sl, :], rkgT[:, jb, gcol:gcol + 64], S_bf[:, jb, :],
                                     start=(jb == 0), stop=False)
                nc.tensor.matmul(ops[sl, :], inT[sl, p * 128 + poff:p * 128 + poff + 64],
                                 vnew[sl, :], start=False, stop=True)
                nc.vector.tensor_copy(o_nat[sl, tt, :], ops[sl, :])
                for jb in range(2):
                    sup = ps_s.tile([128, d], F32, tag="s")
                    nc.tensor.matmul(sup[:], wkgN[sl, tt, jb * 128:(jb + 1) * 128],
                                     vnew[sl, :])
                    nc.vector.scalar_tensor_tensor(
                        S_bf[:, jb, :], S_bf[:, jb, :], gcv_s[:], sup[:],
                        mybir.AluOpType.mult, mybir.AluOpType.add)
            if os.environ.get("K_STOP") == "p3":
                continue
            # ---------------- P4 ----------------
            oT = qbuf.tile([128, 2, QT], BF16, tag="oT")
            for p in range(QPR):
                for kt in range(2):
                    otp = ps_g.tile([128, 128], BF16, tag="g")
                    nc.tensor.transpose(otp[:], o_nat[:, p, kt * 128:(kt + 1) * 128], id_s[:])
                    nc.vector.tensor_copy(oT[:, kt, p * 128:(p + 1) * 128], otp[:])
                st = stage.tile([128, D], F32, tag="st")
                for nh in range(2):
                    pps = ps_p.tile([128, 512], F32, tag="p")
                    for kt in range(2):
                        nc.tensor.matmul(pps[:], oT[:, kt, p * 128:(p + 1) * 128],
                                         wrt_s[:, kt, nh * 512:(nh + 1) * 512],
                                         start=(kt == 0), stop=(kt == 1))
                    nc.vector.tensor_copy(st[:, nh * 512:(nh + 1) * 512], pps[:])
                nc.sync.dma_start(
                    part_d.ap()[qt0 + p * 128: qt0 + (p + 1) * 128, :], st[:])
    nc.compile()
    return nc


_NC = None
LAST_EXEC_NS = None
LAST_TRACE = None


def _bf16(a):
    return np.ascontiguousarray(a.astype(ml_dtypes.bfloat16))


def kernel(out, Ww, Wr, decay, log_alpha):
    global _NC
    out = np.asarray(out, dtype=np.float32)
    Ww = np.asarray(Ww, dtype=np.float32)
    Wr = np.asarray(Wr, dtype=np.float32)
    decay = np.asarray(decay, dtype=np.float32)
    log_alpha = np.asarray(log_alpha, dtype=np.float32)
    gamma = 1.0 / (1.0 + np.exp(-decay.astype(np.float64)))
    alpha = np.exp(log_alpha.astype(np.float64))

    if _NC is None:
        _NC = _build()
    nc = _NC

    p64 = np.arange(64)
    in_maps = []
    for c in range(8):
        b, h = c // 4, c % 4
        g = gamma[h]
        # x with head-h channels rotated to the front so the kernel's
        # xh slice [:, 0:d] is the head slice (v-proj uses matching
        # rotated WwT so the product is unchanged).
        xr = np.roll(out[b], -h * d, axis=1)
        wwr = np.roll(Ww[h * d:(h + 1) * d, :], -h * d, axis=1).T  # (D, d)
        wrs = (alpha[h] * Wr[:, h * d:(h + 1) * d]).T              # (d, D)
        Ls = np.tril(g ** np.maximum(p64[:, None] - p64[None, :], 0), -1)
        mbB = (-Ls).astype(np.float32)
        mitB = np.triu(g ** np.maximum(p64[None, :] - p64[:, None], 0), 1).astype(np.float32)
        z = np.zeros((64, 64), np.float32)
        mb = np.block([[mbB, z], [z, mbB]])
        mit = np.block([[mitB, z], [z, mitB]])
        gp = (g ** p64).astype(np.float32)
        gpb = np.tile(gp, QT // 64)[None, :].repeat(128, 0)
        gpt = (g ** (63 - (np.arange(128) % 64)))[:, None].astype(np.float32)
        gcv = np.full((128, 1), g ** 64, np.float32)
        in_maps.append({
            "xbf": _bf16(xr),
            "wwt": _bf16(wwr),
            "wrt": _bf16(wrs),
            "mb": mb, "mc": np.ascontiguousarray(mb.T),
            "mit": mit,
            "ident": _bf16(np.eye(128, dtype=np.float32)),
            "gpbf": _bf16(gpb),
            "gpt": gpt, "gcv": gcv,
        })

    ncore = int(os.environ.get("K_NCORES", "8"))
    res = bass_utils.run_bass_kernel_spmd(
        nc, in_maps[:ncore], core_ids=list(range(ncore)),
        trace=bool(os.environ.get("K_TRACE")))
    global LAST_EXEC_NS, LAST_TRACE
    LAST_EXEC_NS = res.exec_time_ns
    LAST_TRACE = res.instructions_and_trace
    final = out.copy()
    for c in range(len(res.results)):
        b = c // 4
        final[b] += res.results[c]["partial"]
    return final


# revision 15
# speedup vs baseline: 1.0553x; 1.0516x over previous
"""DeltaHebbianBlock Trainium2 kernel.

Sharding: 8 cores = (B=2) x (H=4) head-parallel. Each core computes its
head's delta-rule chunked scan and the partial output projection
partial_bh = (alpha_h * o_bh) @ Wr_h^T  (8192 x 1024).
Host gathers: out[b] = x[b] + sum_h partial[b,h].

Per-core pipeline (T=8192, d=256, C=64, 128 chunks, 8 quarter-passes):
  P1: DMA-transpose x -> xT (bf16), v = x @ WwT (bf16 mm, f32 psum),
      rk = normalize(x_h), rkT via PE transpose, wk = shift(rk) via SBUF DMA.
  P2: per chunk-pair (block-diag 128x128): grams W = wk wk^T, intraT;
      A^T = (I+C0)(I+C1)(I+C2) truncated nilpotent chain (exact to A0^7);
      v_corr = A v, wk_corrT = (A wk)^T; rkgT, wkgN scalings.
  P3: sequential scan: v_new = v_corr - wk_corr S; o = rkg S + intra v_new;
      S = gC S + wkgN^T v_new.
  P4: oT via PE transpose; partial = oT^T @ (alpha WrT) (bf16 mm).
"""
import os
import numpy as np
import ml_dtypes
from contextlib import ExitStack

import concourse.bass as bass
import concourse.mybir as mybir
import concourse.tile as tile
from concourse import bacc, bass_utils

B, T, D = 2, 8192, 1024
H, d, C = 4, 256, 64
NCH = T // C          # 128 chunks
NQ = 8                # quarter passes
QT = T // NQ          # 1024 tokens per pass
QTT = QT // 128       # 8 p-tiles per pass
QCH = QT // C         # 16 chunks per pass
QPR = QCH // 2        # 8 pairs per pass

F32 = mybir.dt.float32
BF16 = mybir.dt.bfloat16


def _build():
    nc = bacc.Bacc("TRN2", target_bir_lowering=False, debug=False, num_devices=int(os.environ.get("K_NCORES", "8")))
    xbf = nc.dram_tensor("xbf", (T, D), BF16, kind="ExternalInput")
    wwt = nc.dram_tensor("wwt", (D, d), BF16, kind="ExternalInput")
    wrt = nc.dram_tensor("wrt", (d, D), BF16, kind="ExternalInput")
    mb_d = nc.dram_tensor("mb", (128, 128), F32, kind="ExternalInput")
    mc_d = nc.dram_tensor("mc", (128, 128), F32, kind="ExternalInput")
    mit_d = nc.dram_tensor("mit", (128, 128), F32, kind="ExternalInput")
    id_d = nc.dram_tensor("ident", (128, 128), BF16, kind="ExternalInput")
    gpb_d = nc.dram_tensor("gpbf", (128, QT), BF16, kind="ExternalInput")
    gpt_d = nc.dram_tensor("gpt", (128, 1), F32, kind="ExternalInput")
    gcv_d = nc.dram_tensor("gcv", (128, 1), F32, kind="ExternalInput")
    part_d = nc.dram_tensor("partial", (T, D), F32, kind="ExternalOutput")

    with ExitStack() as ctx:
        tc = ctx.enter_context(tile.TileContext(nc))
        consts = ctx.enter_context(tc.tile_pool(name="consts", bufs=1))
        big = ctx.enter_context(tc.tile_pool(name="big", bufs=1))
        qbuf = ctx.enter_context(tc.tile_pool(name="qbuf", bufs=1))
        qbuf2 = ctx.enter_context(tc.tile_pool(name="qbuf2", bufs=2))
        chain = ctx.enter_context(tc.tile_pool(name="chain", bufs=2))
        vnewp = ctx.enter_context(tc.tile_pool(name="vnewp", bufs=3))
        stage = ctx.enter_context(tc.tile_pool(name="stage", bufs=3))
        scr = ctx.enter_context(tc.tile_pool(name="scr", bufs=2))
        ps_g = ctx.enter_context(tc.tile_pool(name="ps_g", bufs=2, space="PSUM"))
        ps_a = ctx.enter_context(tc.tile_pool(name="ps_a", bufs=2, space="PSUM"))
        ps_s = ctx.enter_context(tc.tile_pool(name="ps_s", bufs=2, space="PSUM"))
        ps_p = ctx.enter_context(tc.tile_pool(name="ps_p", bufs=2, space="PSUM"))

        # ---- constants / weights in SBUF ----
        wwt_s = consts.tile([128, 8, d], BF16)
        nc.sync.dma_start(wwt_s[:], wwt.ap().rearrange("(kb p) j -> p kb j", p=128))
        wrt_s = consts.tile([128, 2, D], BF16)
        nc.sync.dma_start(wrt_s[:], wrt.ap().rearrange("(kt p) n -> p kt n", p=128))
        mb_s = consts.tile([128, 128], F32)
        nc.sync.dma_start(mb_s[:], mb_d.ap())
        mc_s = consts.tile([128, 128], F32)
        nc.sync.dma_start(mc_s[:], mc_d.ap())
        mit_s = consts.tile([128, 128], F32)
        nc.sync.dma_start(mit_s[:], mit_d.ap())
        id_s = consts.tile([128, 128], BF16)
        nc.sync.dma_start(id_s[:], id_d.ap())
        gpb_s = consts.tile([128, QT], BF16)
        nc.sync.dma_start(gpb_s[:], gpb_d.ap())
        gpt_s = consts.tile([128, 1], F32)
        nc.sync.dma_start(gpt_s[:], gpt_d.ap())
        gcv_s = consts.tile([128, 1], F32)
        nc.sync.dma_start(gcv_s[:], gcv_d.ap())

        # ---- full-T persistent (bf16) ----
        rk = big.tile([128, T // 128, d], BF16)       # 4MB
        wk = big.tile([128, T // 128, d], BF16)       # 4MB
        rkT = big.tile([128, 2, T + 1], BF16)         # 4MB (col 0 = zero pad)
        S_bf = big.tile([128, 2, d], BF16)
        nc.gpsimd.memset(S_bf[:], 0.0)
        nc.gpsimd.memset(rkT[:, :, 0:1], 0.0)
        nc.gpsimd.memset(wk[0:1, 0:1, :], 0.0)

        for q in range(NQ):
            if os.environ.get("K_STOP") == "consts":
                break
            qt0 = q * QT          # token offset
            tt0 = q * QTT         # p-tile offset
            # ---------------- P1 ----------------
            xT = qbuf2.tile([128, 8, QT], BF16, tag="xT")
            for kb in range(8):
                nc.sync.dma_start(
                    xT[:, kb, :],
                    xbf.ap()[qt0:qt0 + QT, kb * 128:(kb + 1) * 128],
                    transpose=True)
            if os.environ.get("K_STOP") == "xt":
                continue
            xh = qbuf.tile([128, QTT, d], BF16, tag="xh")
            h_ap = xbf.ap()[qt0:qt0 + QT, :]  # head slice set on host via col offset 0
            nc.sync.dma_start(
                xh[:], h_ap[:, 0:d].rearrange("(tt p) j -> p tt j", p=128))
            if os.environ.get("K_STOP") == "xh":
                continue
            v_nat = qbuf2.tile([128, QTT, d], BF16, tag="v_nat")
            for tt in range(QTT):
                vps = ps_p.tile([128, d], F32, tag="p")
                nkb = int(os.environ.get("K_KB", "8"))
                for kb in range(nkb):
                    nc.tensor.matmul(vps[:], xT[:, kb, tt * 128:(tt + 1) * 128],
                                     wwt_s[:, kb, :], start=(kb == 0), stop=(kb == nkb - 1))
                nc.vector.tensor_copy(v_nat[:, tt, :], vps[:])
            if os.environ.get("K_STOP") == "v":
                continue
            # rk = normalize(xh)
            rklvl = os.environ.get("K_RK", "all")
            for tt in range(QTT):
                sq = scr.tile([128, d], F32, tag="sq")
                ss = scr.tile([128, 1], F32, tag="ss")
                nc.scalar.activation(sq[:], xh[:, tt, :],
                                     mybir.ActivationFunctionType.Square,
                                     accum_out=ss[:])
                if rklvl == "red":
                    continue
                nrm = scr.tile([128, 1], F32, tag="nrm")
                nc.scalar.activation(nrm[:], ss[:], mybir.ActivationFunctionType.Sqrt)
                inv = scr.tile([128, 1], F32, tag="inv")
                nc.vector.reciprocal(inv[:], nrm[:])
                if rklvl == "sqrt":
                    continue
                nc.scalar.activation(rk[:, tt0 + tt, :], xh[:, tt, :],
                                     mybir.ActivationFunctionType.Copy, scale=inv[:])
                if rklvl == "scale":
                    continue
                for kt in range(2):
                    tps = ps_g.tile([128, 128], BF16, tag="g")
                    nc.tensor.transpose(tps[:], rk[:, tt0 + tt, kt * 128:(kt + 1) * 128],
                                        id_s[:])
                    nc.vector.tensor_copy(
                        rkT[:, kt, 1 + qt0 + tt * 128: 1 + qt0 + (tt + 1) * 128], tps[:])
            if os.environ.get("K_STOP") == "rk":
                continue
            # wk = shift(rk) by one row
            nc.sync.dma_start(wk[1:128, tt0:tt0 + QTT, :], rk[0:127, tt0:tt0 + QTT, :])
            lo = max(tt0, 1)
            nc.sync.dma_start(wk[0:1, lo:tt0 + QTT, :], rk[127:128, lo - 1:tt0 + QTT - 1, :])
            # wkgN = wk * gp_tail (per-partition), rkgT = rkT * gp (per-col)
            wkgN = qbuf.tile([128, QTT, d], BF16, tag="wkgN")
            nc.scalar.activation(wkgN[:], wk[:, tt0:tt0 + QTT, :],
                                 mybir.ActivationFunctionType.Copy, scale=gpt_s[:])
            rkgT = qbuf.tile([128, 2, QT], BF16, tag="rkgT")
            for kt in range(2):
                nc.vector.tensor_mul(rkgT[:, kt, :], rkT[:, kt, 1 + qt0:1 + qt0 + QT],
                                     gpb_s[:])
            if os.environ.get("K_STOP") == "p1":
                continue
            # ---------------- P2 ----------------
            AT = qbuf.tile([128, QPR * 128], BF16, tag="AT")
            inT = qbuf.tile([128, QPR * 128], BF16, tag="inT")
            v_corr = qbuf.tile([128, QTT, d], BF16, tag="v_corr")
            wkcT = qbuf.tile([128, 2, QT], BF16, tag="wkcT")
            for p in range(QPR):
                w0 = qt0 + p * 128  # global token col of pair window
                gps = ps_g.tile([128, 128], F32, tag="g")
                for kt in range(2):
                    nc.tensor.matmul(gps[:], rkT[:, kt, w0:w0 + 128],
                                     rkT[:, kt, w0:w0 + 128],
                                     start=(kt == 0), stop=(kt == 1))
                B0 = chain.tile([128, 128], BF16, tag="B0")
                nc.vector.tensor_mul(B0[:], gps[:], mb_s[:])
                C0 = chain.tile([128, 128], BF16, tag="C0")
                nc.vector.tensor_mul(C0[:], gps[:], mc_s[:])
                ips = ps_g.tile([128, 128], F32, tag="g")
                for kt in range(2):
                    nc.tensor.matmul(ips[:], rkT[:, kt, w0:w0 + 128],
                                     rkT[:, kt, w0 + 1:w0 + 129],
                                     start=(kt == 0), stop=(kt == 1))
                nc.vector.tensor_mul(inT[:, p * 128:(p + 1) * 128], ips[:], mit_s[:])
                # chain: C1 = C0^2, B1 = C1^T-path, C2 = C1^2
                c1p = ps_g.tile([128, 128], F32, tag="g")
                nc.tensor.matmul(c1p[:], B0[:], C0[:])
                C1 = chain.tile([128, 128], BF16, tag="C1")
                nc.vector.tensor_copy(C1[:], c1p[:])
                b1p = ps_g.tile([128, 128], F32, tag="g")
                nc.tensor.matmul(b1p[:], C0[:], B0[:])
                B1 = chain.tile([128, 128], BF16, tag="B1")
                nc.vector.tensor_copy(B1[:], b1p[:])
                c2p = ps_g.tile([128, 128], F32, tag="g")
                nc.tensor.matmul(c2p[:], B1[:], C1[:])
                C2 = chain.tile([128, 128], BF16, tag="C2")
                nc.vector.tensor_copy(C2[:], c2p[:])
                G0 = chain.tile([128, 128], BF16, tag="G0")
                nc.vector.tensor_add(G0[:], B0[:], id_s[:])
                Gh0 = chain.tile([128, 128], BF16, tag="Gh0")
                nc.vector.tensor_add(Gh0[:], C0[:], id_s[:])
                g1p = ps_g.tile([128, 128], F32, tag="g")
                nc.tensor.matmul(g1p[:], G0[:], C1[:])
                G1h = chain.tile([128, 128], BF16, tag="G1h")
                nc.vector.tensor_add(G1h[:], g1p[:], Gh0[:])
                g1tp = ps_g.tile([128, 128], BF16, tag="g")
                nc.tensor.transpose(g1tp[:], G1h[:], id_s[:])
                G1 = chain.tile([128, 128], BF16, tag="G1")
                nc.vector.tensor_copy(G1[:], g1tp[:])
                g2p = ps_g.tile([128, 128], F32, tag="g")
                nc.tensor.matmul(g2p[:], G1[:], C2[:])
                nc.vector.tensor_add(AT[:, p * 128:(p + 1) * 128], g2p[:], G1h[:])
                # applications
                vcp = ps_a.tile([128, d], F32, tag="a")
                nc.tensor.matmul(vcp[:], AT[:, p * 128:(p + 1) * 128], v_nat[:, p, :])
                nc.vector.tensor_copy(v_corr[:, p, :], vcp[:])
                for jb in range(2):
                    wcp = ps_a.tile([128, 128], F32, tag="a")
                    nc.tensor.matmul(wcp[:], wk[:, tt0 + p, jb * 128:(jb + 1) * 128],
                                     AT[:, p * 128:(p + 1) * 128])
                    nc.vector.tensor_copy(wkcT[:, jb, p * 128:(p + 1) * 128], wcp[:])
            if os.environ.get("K_STOP") == "p2":
                continue
            # ---------------- P3: scan ----------------
            o_nat = qbuf2.tile([128, QTT, d], BF16, tag="o_nat")
            for cq in range(QCH):
                tt = cq // 2
                poff = (cq % 2) * 64
                gcol = cq * 64
                p = cq // 2
                sl = slice(poff, poff + 64)
                vnp = ps_a.tile([128, d], F32, tag="a")
                for jb in range(2):
                    nc.tensor.matmul(vnp[sl, :], wkcT[:, jb, gcol:gcol + 64], S_bf[:, jb, :],
                                     start=(jb == 0), stop=(jb == 1))
                vnew = vnewp.tile([128, d], BF16, tag="vnew")
                nc.vector.scalar_tensor_tensor(
                    vnew[sl, :], vnp[sl, :], -1.0, v_corr[sl, tt, :],
                    mybir.AluOpType.mult, mybir.AluOpType.add)
                ops = ps_a.tile([128, d], F32, tag="a")
                for jb in range(2):
                    nc.tensor.matmul(ops[sl, :], rkgT[:, jb, gcol:gcol + 64], S_bf[:, jb, :],
                                     start=(jb == 0), stop=False)
                nc.tensor.matmul(ops[sl, :], inT[sl, p * 128 + poff:p * 128 + poff + 64],
                                 vnew[sl, :], start=False, stop=True)
                nc.scalar.activation(o_nat[sl, tt, :], ops[sl, :],
                                     mybir.ActivationFunctionType.Copy)
                sup = ps_s.tile([128, 2 * d], F32, tag="s")
                for jb in range(2):
                    nc.tensor.matmul(sup[:, jb * d:(jb + 1) * d],
                                     wkgN[sl, tt, jb * 128:(jb + 1) * 128],
                                     vnew[sl, :])
                nc.vector.scalar_tensor_tensor(
                    S_bf[:, :, :], S_bf[:, :, :], gcv_s[:],
                    sup[:].rearrange("p (jb n) -> p jb n", jb=2),
                    mybir.AluOpType.mult, mybir.AluOpType.add)
            if os.environ.get("K_STOP") == "p3":
                continue
            # ---------------- P4 ----------------
            oT = qbuf.tile([128, 2, QT], BF16, tag="oT")
            for p in range(QPR):
                for kt in range(2):
                    otp = ps_g.tile([128, 128], BF16, tag="g")
                    nc.tensor.transpose(otp[:], o_nat[:, p, kt * 128:(kt + 1) * 128], id_s[:])
                    nc.vector.tensor_copy(oT[:, kt, p * 128:(p + 1) * 128], otp[:])
                st = stage.tile([128, D], F32, tag="st")
                for nh in range(2):
                    pps = ps_p.tile([128, 512], F32, tag="p")
                    for kt in range(2):
                        nc.tensor.matmul(pps[:], oT[:, kt, p * 128:(p + 1) * 128],
                                         wrt_s[:, kt, nh * 512:(nh + 1) * 512],
                                         start=(kt == 0), stop=(kt == 1))
                    nc.vector.tensor_copy(st[:, nh * 512:(nh + 1) * 512], pps[:])
                nc.sync.dma_start(
                    part_d.ap()[qt0 + p * 128: qt0 + (p + 1) * 128, :], st[:])
    nc.compile()
    return nc


_NC = None
LAST_EXEC_NS = None
LAST_TRACE = None


def _bf16(a):
    return np.ascontiguousarray(a.astype(ml_dtypes.bfloat16))


def kernel(out, Ww, Wr, decay, log_alpha):
    global _NC
    out = np.asarray(out, dtype=np.float32)
    Ww = np.asarray(Ww, dtype=np.float32)
    Wr = np.asarray(Wr, dtype=np.float32)
    decay = np.asarray(decay, dtype=np.float32)
    log_alpha = np.asarray(log_alpha, dtype=np.float32)
    gamma = 1.0 / (1.0 + np.exp(-decay.astype(np.float64)))
    alpha = np.exp(log_alpha.astype(np.float64))

    if _NC is None:
        _NC = _build()
    nc = _NC

    p64 = np.arange(64)
    in_maps = []
    for c in range(8):
        b, h = c // 4, c % 4
        g = gamma[h]
        # x with head-h channels rotated to the front so the kernel's
        # xh slice [:, 0:d] is the head slice (v-proj uses matching
        # rotated WwT so the product is unchanged).
        xr = np.roll(out[b], -h * d, axis=1)
        wwr = np.roll(Ww[h * d:(h + 1) * d, :], -h * d, axis=1).T  # (D, d)
        wrs = (alpha[h] * Wr[:, h * d:(h + 1) * d]).T              # (d, D)
        Ls = np.tril(g ** np.maximum(p64[:, None] - p64[None, :], 0), -1)
        mbB = (-Ls).astype(np.float32)
        mitB = np.triu(g ** np.maximum(p64[None, :] - p64[:, None], 0), 1).astype(np.float32)
        z = np.zeros((64, 64), np.float32)
        mb = np.block([[mbB, z], [z, mbB]])
        mit = np.block([[mitB, z], [z, mitB]])
        gp = (g ** p64).astype(np.float32)
        gpb = np.tile(gp, QT // 64)[None, :].repeat(128, 0)
        gpt = (g ** (63 - (np.arange(128) % 64)))[:, None].astype(np.float32)
        gcv = np.full((128, 1), g ** 64, np.float32)
        in_maps.append({
            "xbf": _bf16(xr),
            "wwt": _bf16(wwr),
            "wrt": _bf16(wrs),
            "mb": mb, "mc": np.ascontiguousarray(mb.T),
            "mit": mit,
            "ident": _bf16(np.eye(128, dtype=np.float32)),
            "gpbf": _bf16(gpb),
            "gpt": gpt, "gcv": gcv,
        })

    ncore = int(os.environ.get("K_NCORES", "8"))
    res = bass_utils.run_bass_kernel_spmd(
        nc, in_maps[:ncore], core_ids=list(range(ncore)),
        trace=bool(os.environ.get("K_TRACE")))
    global LAST_EXEC_NS, LAST_TRACE
    LAST_EXEC_NS = res.exec_time_ns
    LAST_TRACE = res.instructions_and_trace
    final = out.copy()
    for c in range(len(res.results)):
        b = c // 4
        final[b] += res.results[c]["partial"]
    return final


# revision 16
# speedup vs baseline: 1.0686x; 1.0127x over previous
"""DeltaHebbianBlock Trainium2 kernel.

Sharding: 8 cores = (B=2) x (H=4) head-parallel. Each core computes its
head's delta-rule chunked scan and the partial output projection
partial_bh = (alpha_h * o_bh) @ Wr_h^T  (8192 x 1024).
Host gathers: out[b] = x[b] + sum_h partial[b,h].

Per-core pipeline (T=8192, d=256, C=64, 128 chunks, 8 quarter-passes):
  P1: DMA-transpose x -> xT (bf16), v = x @ WwT (bf16 mm, f32 psum),
      rk = normalize(x_h), rkT via PE transpose, wk = shift(rk) via SBUF DMA.
  P2: per chunk-pair (block-diag 128x128): grams W = wk wk^T, intraT;
      A^T = (I+C0)(I+C1)(I+C2) truncated nilpotent chain (exact to A0^7);
      v_corr = A v, wk_corrT = (A wk)^T; rkgT, wkgN scalings.
  P3: sequential scan: v_new = v_corr - wk_corr S; o = rkg S + intra v_new;
      S = gC S + wkgN^T v_new.
  P4: oT via PE transpose; partial = oT^T @ (alpha WrT) (bf16 mm).
"""
import os
import numpy as np
import ml_dtypes
from contextlib import ExitStack

import concourse.bass as bass
import concourse.mybir as mybir
import concourse.tile as tile
from concourse import bacc, bass_utils

B, T, D = 2, 8192, 1024
H, d, C = 4, 256, 64
NCH = T // C          # 128 chunks
NQ = 8                # quarter passes
QT = T // NQ          # 1024 tokens per pass
QTT = QT // 128       # 8 p-tiles per pass
QCH = QT // C         # 16 chunks per pass
QPR = QCH // 2        # 8 pairs per pass

F32 = mybir.dt.float32
BF16 = mybir.dt.bfloat16


def _build():
    nc = bacc.Bacc("TRN2", target_bir_lowering=False, debug=False, num_devices=int(os.environ.get("K_NCORES", "8")))
    xbf = nc.dram_tensor("xbf", (T, D), BF16, kind="ExternalInput")
    wwt = nc.dram_tensor("wwt", (D, d), BF16, kind="ExternalInput")
    wrt = nc.dram_tensor("wrt", (d, D), BF16, kind="ExternalInput")
    mb_d = nc.dram_tensor("mb", (128, 128), F32, kind="ExternalInput")
    mc_d = nc.dram_tensor("mc", (128, 128), F32, kind="ExternalInput")
    mit_d = nc.dram_tensor("mit", (128, 128), F32, kind="ExternalInput")
    id_d = nc.dram_tensor("ident", (128, 128), BF16, kind="ExternalInput")
    gpb_d = nc.dram_tensor("gpbf", (128, QT), BF16, kind="ExternalInput")
    gpt_d = nc.dram_tensor("gpt", (128, 1), F32, kind="ExternalInput")
    gcv_d = nc.dram_tensor("gcv", (128, 1), F32, kind="ExternalInput")
    part_d = nc.dram_tensor("partial", (T, D), F32, kind="ExternalOutput")

    with ExitStack() as ctx:
        tc = ctx.enter_context(tile.TileContext(nc))
        consts = ctx.enter_context(tc.tile_pool(name="consts", bufs=1))
        big = ctx.enter_context(tc.tile_pool(name="big", bufs=1))
        qbuf = ctx.enter_context(tc.tile_pool(name="qbuf", bufs=1))
        qbuf2 = ctx.enter_context(tc.tile_pool(name="qbuf2", bufs=2))
        chain = ctx.enter_context(tc.tile_pool(name="chain", bufs=3))
        vnewp = ctx.enter_context(tc.tile_pool(name="vnewp", bufs=4))
        stage = ctx.enter_context(tc.tile_pool(name="stage", bufs=3))
        scr = ctx.enter_context(tc.tile_pool(name="scr", bufs=2))
        ps_g = ctx.enter_context(tc.tile_pool(name="ps_g", bufs=2, space="PSUM"))
        ps_a = ctx.enter_context(tc.tile_pool(name="ps_a", bufs=3, space="PSUM"))
        ps_s = ctx.enter_context(tc.tile_pool(name="ps_s", bufs=1, space="PSUM"))
        ps_p = ctx.enter_context(tc.tile_pool(name="ps_p", bufs=2, space="PSUM"))

        # ---- constants / weights in SBUF ----
        wwt_s = consts.tile([128, 8, d], BF16)
        nc.sync.dma_start(wwt_s[:], wwt.ap().rearrange("(kb p) j -> p kb j", p=128))
        wrt_s = consts.tile([128, 2, D], BF16)
        nc.sync.dma_start(wrt_s[:], wrt.ap().rearrange("(kt p) n -> p kt n", p=128))
        mb_s = consts.tile([128, 128], F32)
        nc.sync.dma_start(mb_s[:], mb_d.ap())
        mc_s = consts.tile([128, 128], F32)
        nc.sync.dma_start(mc_s[:], mc_d.ap())
        mit_s = consts.tile([128, 128], F32)
        nc.sync.dma_start(mit_s[:], mit_d.ap())
        id_s = consts.tile([128, 128], BF16)
        nc.sync.dma_start(id_s[:], id_d.ap())
        gpb_s = consts.tile([128, QT], BF16)
        nc.sync.dma_start(gpb_s[:], gpb_d.ap())
        gpt_s = consts.tile([128, 1], F32)
        nc.sync.dma_start(gpt_s[:], gpt_d.ap())
        gcv_s = consts.tile([128, 1], F32)
        nc.sync.dma_start(gcv_s[:], gcv_d.ap())

        # ---- full-T persistent (bf16) ----
        rk = big.tile([128, T // 128, d], BF16)       # 4MB
        wk = big.tile([128, T // 128, d], BF16)       # 4MB
        rkT = big.tile([128, 2, T + 1], BF16)         # 4MB (col 0 = zero pad)
        S_bf = big.tile([128, 2, d], BF16)
        nc.gpsimd.memset(S_bf[:], 0.0)
        nc.gpsimd.memset(rkT[:, :, 0:1], 0.0)
        nc.gpsimd.memset(wk[0:1, 0:1, :], 0.0)

        for q in range(NQ):
            if os.environ.get("K_STOP") == "consts":
                break
            qt0 = q * QT          # token offset
            tt0 = q * QTT         # p-tile offset
            # ---------------- P1 ----------------
            xT = qbuf2.tile([128, 8, QT], BF16, tag="xT")
            for kb in range(8):
                nc.sync.dma_start(
                    xT[:, kb, :],
                    xbf.ap()[qt0:qt0 + QT, kb * 128:(kb + 1) * 128],
                    transpose=True)
            if os.environ.get("K_STOP") == "xt":
                continue
            xh = qbuf.tile([128, QTT, d], BF16, tag="xh")
            h_ap = xbf.ap()[qt0:qt0 + QT, :]  # head slice set on host via col offset 0
            nc.sync.dma_start(
                xh[:], h_ap[:, 0:d].rearrange("(tt p) j -> p tt j", p=128))
            if os.environ.get("K_STOP") == "xh":
                continue
            v_nat = qbuf2.tile([128, QTT, d], BF16, tag="v_nat")
            for tt in range(QTT):
                vps = ps_p.tile([128, d], F32, tag="p")
                nkb = int(os.environ.get("K_KB", "8"))
                for kb in range(nkb):
                    nc.tensor.matmul(vps[:], xT[:, kb, tt * 128:(tt + 1) * 128],
                                     wwt_s[:, kb, :], start=(kb == 0), stop=(kb == nkb - 1))
                nc.vector.tensor_copy(v_nat[:, tt, :], vps[:])
            if os.environ.get("K_STOP") == "v":
                continue
            # rk = normalize(xh)
            rklvl = os.environ.get("K_RK", "all")
            for tt in range(QTT):
                sq = scr.tile([128, d], F32, tag="sq")
                ss = scr.tile([128, 1], F32, tag="ss")
                nc.scalar.activation(sq[:], xh[:, tt, :],
                                     mybir.ActivationFunctionType.Square,
                                     accum_out=ss[:])
                if rklvl == "red":
                    continue
                nrm = scr.tile([128, 1], F32, tag="nrm")
                nc.scalar.activation(nrm[:], ss[:], mybir.ActivationFunctionType.Sqrt)
                inv = scr.tile([128, 1], F32, tag="inv")
                nc.vector.reciprocal(inv[:], nrm[:])
                if rklvl == "sqrt":
                    continue
                nc.scalar.activation(rk[:, tt0 + tt, :], xh[:, tt, :],
                                     mybir.ActivationFunctionType.Copy, scale=inv[:])
                if rklvl == "scale":
                    continue
                for kt in range(2):
                    tps = ps_g.tile([128, 128], BF16, tag="g")
                    nc.tensor.transpose(tps[:], rk[:, tt0 + tt, kt * 128:(kt + 1) * 128],
                                        id_s[:])
                    nc.vector.tensor_copy(
                        rkT[:, kt, 1 + qt0 + tt * 128: 1 + qt0 + (tt + 1) * 128], tps[:])
            if os.environ.get("K_STOP") == "rk":
                continue
            # wk = shift(rk) by one row
            nc.sync.dma_start(wk[1:128, tt0:tt0 + QTT, :], rk[0:127, tt0:tt0 + QTT, :])
            lo = max(tt0, 1)
            nc.sync.dma_start(wk[0:1, lo:tt0 + QTT, :], rk[127:128, lo - 1:tt0 + QTT - 1, :])
            # wkgN = wk * gp_tail (per-partition), rkgT = rkT * gp (per-col)
            wkgN = qbuf.tile([128, QTT, d], BF16, tag="wkgN")
            nc.scalar.activation(wkgN[:], wk[:, tt0:tt0 + QTT, :],
                                 mybir.ActivationFunctionType.Copy, scale=gpt_s[:])
            rkgT = qbuf.tile([128, 2, QT], BF16, tag="rkgT")
            for kt in range(2):
                nc.vector.tensor_mul(rkgT[:, kt, :], rkT[:, kt, 1 + qt0:1 + qt0 + QT],
                                     gpb_s[:])
            if os.environ.get("K_STOP") == "p1":
                continue
            # ---------------- P2 ----------------
            AT = qbuf.tile([128, QPR * 128], BF16, tag="AT")
            inT = qbuf.tile([128, QPR * 128], BF16, tag="inT")
            v_corr = qbuf.tile([128, QTT, d], BF16, tag="v_corr")
            wkcT = qbuf.tile([128, 2, QT], BF16, tag="wkcT")
            for p in range(QPR):
                w0 = qt0 + p * 128  # global token col of pair window
                gps = ps_g.tile([128, 128], F32, tag="g")
                for kt in range(2):
                    nc.tensor.matmul(gps[:], rkT[:, kt, w0:w0 + 128],
                                     rkT[:, kt, w0:w0 + 128],
                                     start=(kt == 0), stop=(kt == 1))
                B0 = chain.tile([128, 128], BF16, tag="B0")
                nc.vector.tensor_mul(B0[:], gps[:], mb_s[:])
                C0 = chain.tile([128, 128], BF16, tag="C0")
                nc.vector.tensor_mul(C0[:], gps[:], mc_s[:])
                ips = ps_g.tile([128, 128], F32, tag="g")
                for kt in range(2):
                    nc.tensor.matmul(ips[:], rkT[:, kt, w0:w0 + 128],
                                     rkT[:, kt, w0 + 1:w0 + 129],
                                     start=(kt == 0), stop=(kt == 1))
                nc.vector.tensor_mul(inT[:, p * 128:(p + 1) * 128], ips[:], mit_s[:])
                # chain: C1 = C0^2, B1 = C1^T-path, C2 = C1^2
                c1p = ps_g.tile([128, 128], F32, tag="g")
                nc.tensor.matmul(c1p[:], B0[:], C0[:])
                C1 = chain.tile([128, 128], BF16, tag="C1")
                nc.vector.tensor_copy(C1[:], c1p[:])
                b1p = ps_g.tile([128, 128], F32, tag="g")
                nc.tensor.matmul(b1p[:], C0[:], B0[:])
                B1 = chain.tile([128, 128], BF16, tag="B1")
                nc.vector.tensor_copy(B1[:], b1p[:])
                c2p = ps_g.tile([128, 128], F32, tag="g")
                nc.tensor.matmul(c2p[:], B1[:], C1[:])
                C2 = chain.tile([128, 128], BF16, tag="C2")
                nc.vector.tensor_copy(C2[:], c2p[:])
                G0 = chain.tile([128, 128], BF16, tag="G0")
                nc.vector.tensor_add(G0[:], B0[:], id_s[:])
                Gh0 = chain.tile([128, 128], BF16, tag="Gh0")
                nc.vector.tensor_add(Gh0[:], C0[:], id_s[:])
                g1p = ps_g.tile([128, 128], F32, tag="g")
                nc.tensor.matmul(g1p[:], G0[:], C1[:])
                G1h = chain.tile([128, 128], BF16, tag="G1h")
                nc.vector.tensor_add(G1h[:], g1p[:], Gh0[:])
                g1tp = ps_g.tile([128, 128], BF16, tag="g")
                nc.tensor.transpose(g1tp[:], G1h[:], id_s[:])
                G1 = chain.tile([128, 128], BF16, tag="G1")
                nc.vector.tensor_copy(G1[:], g1tp[:])
                g2p = ps_g.tile([128, 128], F32, tag="g")
                nc.tensor.matmul(g2p[:], G1[:], C2[:])
                nc.vector.tensor_add(AT[:, p * 128:(p + 1) * 128], g2p[:], G1h[:])
                # applications
                vcp = ps_a.tile([128, d], F32, tag="a")
                nc.tensor.matmul(vcp[:], AT[:, p * 128:(p + 1) * 128], v_nat[:, p, :])
                nc.vector.tensor_copy(v_corr[:, p, :], vcp[:])
                for jb in range(2):
                    wcp = ps_a.tile([128, 128], F32, tag="a")
                    nc.tensor.matmul(wcp[:], wk[:, tt0 + p, jb * 128:(jb + 1) * 128],
                                     AT[:, p * 128:(p + 1) * 128])
                    nc.vector.tensor_copy(wkcT[:, jb, p * 128:(p + 1) * 128], wcp[:])
            if os.environ.get("K_STOP") == "p2":
                continue
            # ---------------- P3: scan ----------------
            o_nat = qbuf2.tile([128, QTT, d], BF16, tag="o_nat")
            for cq in range(QCH):
                tt = cq // 2
                poff = (cq % 2) * 64
                gcol = cq * 64
                p = cq // 2
                sl = slice(poff, poff + 64)
                vnp = ps_a.tile([128, d], F32, tag="a")
                for jb in range(2):
                    nc.tensor.matmul(vnp[sl, :], wkcT[:, jb, gcol:gcol + 64], S_bf[:, jb, :],
                                     start=(jb == 0), stop=(jb == 1))
                vnew = vnewp.tile([128, d], BF16, tag="vnew")
                nc.vector.scalar_tensor_tensor(
                    vnew[sl, :], vnp[sl, :], -1.0, v_corr[sl, tt, :],
                    mybir.AluOpType.mult, mybir.AluOpType.add)
                ops = ps_a.tile([128, d], F32, tag="a")
                for jb in range(2):
                    nc.tensor.matmul(ops[sl, :], rkgT[:, jb, gcol:gcol + 64], S_bf[:, jb, :],
                                     start=(jb == 0), stop=False)
                nc.tensor.matmul(ops[sl, :], inT[sl, p * 128 + poff:p * 128 + poff + 64],
                                 vnew[sl, :], start=False, stop=True)
                nc.scalar.activation(o_nat[sl, tt, :], ops[sl, :],
                                     mybir.ActivationFunctionType.Copy)
                sup = ps_s.tile([128, 2 * d], F32, tag="s")
                for jb in range(2):
                    nc.tensor.matmul(sup[:, jb * d:(jb + 1) * d],
                                     wkgN[sl, tt, jb * 128:(jb + 1) * 128],
                                     vnew[sl, :])
                nc.vector.scalar_tensor_tensor(
                    S_bf[:, :, :], S_bf[:, :, :], gcv_s[:],
                    sup[:].rearrange("p (jb n) -> p jb n", jb=2),
                    mybir.AluOpType.mult, mybir.AluOpType.add)
            if os.environ.get("K_STOP") == "p3":
                continue
            # ---------------- P4 ----------------
            oT = qbuf.tile([128, 2, QT], BF16, tag="oT")
            for p in range(QPR):
                for kt in range(2):
                    otp = ps_g.tile([128, 128], BF16, tag="g")
                    nc.tensor.transpose(otp[:], o_nat[:, p, kt * 128:(kt + 1) * 128], id_s[:])
                    nc.vector.tensor_copy(oT[:, kt, p * 128:(p + 1) * 128], otp[:])
                st = stage.tile([128, D], F32, tag="st")
                for nh in range(2):
                    pps = ps_p.tile([128, 512], F32, tag="p")
                    for kt in range(2):
                        nc.tensor.matmul(pps[:], oT[:, kt, p * 128:(p + 1) * 128],
                                         wrt_s[:, kt, nh * 512:(nh + 1) * 512],
                                         start=(kt == 0), stop=(kt == 1))
                    nc.vector.tensor_copy(st[:, nh * 512:(nh + 1) * 512], pps[:])
                nc.sync.dma_start(
                    part_d.ap()[qt0 + p * 128: qt0 + (p + 1) * 128, :], st[:])
    nc.compile()
    return nc


_NC = None
LAST_EXEC_NS = None
LAST_TRACE = None


def _bf16(a):
    return np.ascontiguousarray(a.astype(ml_dtypes.bfloat16))


def kernel(out, Ww, Wr, decay, log_alpha):
    global _NC
    out = np.asarray(out, dtype=np.float32)
    Ww = np.asarray(Ww, dtype=np.float32)
    Wr = np.asarray(Wr, dtype=np.float32)
    decay = np.asarray(decay, dtype=np.float32)
    log_alpha = np.asarray(log_alpha, dtype=np.float32)
    gamma = 1.0 / (1.0 + np.exp(-decay.astype(np.float64)))
    alpha = np.exp(log_alpha.astype(np.float64))

    if _NC is None:
        _NC = _build()
    nc = _NC

    p64 = np.arange(64)
    in_maps = []
    for c in range(8):
        b, h = c // 4, c % 4
        g = gamma[h]
        # x with head-h channels rotated to the front so the kernel's
        # xh slice [:, 0:d] is the head slice (v-proj uses matching
        # rotated WwT so the product is unchanged).
        xr = np.roll(out[b], -h * d, axis=1)
        wwr = np.roll(Ww[h * d:(h + 1) * d, :], -h * d, axis=1).T  # (D, d)
        wrs = (alpha[h] * Wr[:, h * d:(h + 1) * d]).T              # (d, D)
        Ls = np.tril(g ** np.maximum(p64[:, None] - p64[None, :], 0), -1)
        mbB = (-Ls).astype(np.float32)
        mitB = np.triu(g ** np.maximum(p64[None, :] - p64[:, None], 0), 1).astype(np.float32)
        z = np.zeros((64, 64), np.float32)
        mb = np.block([[mbB, z], [z, mbB]])
        mit = np.block([[mitB, z], [z, mitB]])
        gp = (g ** p64).astype(np.float32)
        gpb = np.tile(gp, QT // 64)[None, :].repeat(128, 0)
        gpt = (g ** (63 - (np.arange(128) % 64)))[:, None].astype(np.float32)
        gcv = np.full((128, 1), g ** 64, np.float32)
        in_maps.append({
            "xbf": _bf16(xr),
            "wwt": _bf16(wwr),
            "wrt": _bf16(wrs),
            "mb": mb, "mc": np.ascontiguousarray(mb.T),
            "mit": mit,
            "ident": _bf16(np.eye(128, dtype=np.float32)),
            "gpbf": _bf16(gpb),
            "gpt": gpt, "gcv": gcv,
        })

    ncore = int(os.environ.get("K_NCORES", "8"))
    res = bass_utils.run_bass_kernel_spmd(
        nc, in_maps[:ncore], core_ids=list(range(ncore)),
        trace=bool(os.environ.get("K_TRACE")))
    global LAST_EXEC_NS, LAST_TRACE
    LAST_EXEC_NS = res.exec_time_ns
    LAST_TRACE = res.instructions_and_trace
    final = out.copy()
    for c in range(len(res.results)):
        b = c // 4
        final[b] += res.results[c]["partial"]
    return final
